# revision 5
# baseline (speedup 1.0000x reference)
"""Trainium2 Bass kernel for nn_GCN_26242250179008.

The reference model is a ChebConv(K=1) stack, which degenerates to plain
dense linear layers (edge_index is never used):

    h = relu(x @ W1.T + b1); h = relu(h @ W2.T + b2); h = h @ W3.T + b3
    g = mean(h, axis=0); out = log_softmax(g @ Wl.T + bl)

Because layer 3 is linear (no relu), mean() commutes with it:
    mean(h3) = mean(h2) @ W3.T + b3
so the device only needs sum_n relu(W2 @ relu(W1 @ x_n + b1) + b2) — a
[128] partial per core.  Layer 3, the classifier head and log_softmax are
O(128^2) and run on host.

Sharding: data-parallel over nodes.  x is split row-wise into 8 shards of
25000 nodes; each shard is transposed on host to [128 features, 25000
nodes] so features sit on SBUF partitions and the matmuls need no
on-device transpose.  The small weights are replicated.  Per-core output
is a [128, 32] fp32 matrix of per-group partial sums; the host reduces
over groups and cores (cheaper than an all-reduce for ~16KB).

Perf notes (measured on HW):
  * Steady state is bound by PSUM evacuation: ScalarE ACTIVATE(relu) ~
    (w+181)/1.2 ns and DVE tensor_scalar(add,max) ~ (w+120)/0.96 ns per
    [128,w] fp32 PSUM group; both engines run in parallel on different
    banks, so relu passes are split between them by accumulated time.
  * Matmuls are [128,128]x[128,1024] bf16, software-pipelined one
    superstep (2 groups) behind layer 1 so the PE never stalls on an
    in-flight relu, with stationary-weight switches batched per phase.
  * The per-group sums go out as one [128, 32] fp32 DMA (128B per
    partition).  A [128,1] output would emit 4-byte descriptors whose
    HBM read-modify-write completion costs ~8us extra at the end.
"""

import math
import os

import numpy as np

N_NODES = 200_000
F = 128
N_CORES = 8
PER_CORE = N_NODES // N_CORES  # 25000

# group widths: two 512 warm-up groups so compute starts on the first
# small DMA chunk, then 1024-wide groups (one PSUM bank pair in fp32).
GROUP_WIDTHS = [512, 512] + [1024] * 23 + [424]
assert sum(GROUP_WIDTHS) == PER_CORE
N_GROUPS = len(GROUP_WIDTHS)  # 26
ACC_COLS = 32  # padded so the out-DMA moves 128B per partition

# DMA chunks (aligned to group boundaries); first ones small so the
# pipeline fills early, the rest ~1MB for full DMA efficiency.
CHUNKS = [512, 512, 1024, 2048, 4096, 4096, 4096, 4096, 4096, 424]
assert sum(CHUNKS) == PER_CORE

# measured per-pass costs for [128,w] fp32 PSUM sources
DVE_NS = lambda w: (w + 120) / 0.96
ACT_NS = lambda w: (w + 181) / 1.2

_COMPILED = {}


def _build_program():
    from concourse import bacc, mybir, tile

    f32 = mybir.dt.float32
    bf16 = mybir.dt.bfloat16

    nc = bacc.Bacc(None, target_bir_lowering=False, debug=False)

    xt = nc.dram_tensor("xt", [F, PER_CORE], bf16, kind="ExternalInput")
    wpk = nc.dram_tensor("wpack", [F, 2 * F], bf16, kind="ExternalInput")
    bpk = nc.dram_tensor("bpack", [F, 2], f32, kind="ExternalInput")
    out = nc.dram_tensor("partial", [F, ACC_COLS], f32, kind="ExternalOutput")

    gstart = []
    pos = 0
    for w in GROUP_WIDTHS:
        gstart.append(pos)
        pos += w

    with tile.TileContext(nc, pool_alloc_mode="queue") as tc:
        with (
            tc.tile_pool(name="const", bufs=1) as cpool,
            tc.tile_pool(name="h1", bufs=4) as h1pool,
            tc.tile_pool(name="ps", bufs=1, space="PSUM") as pspool,
        ):
            wpk_sb = cpool.tile([F, 2 * F], bf16)
            bpk_sb = cpool.tile([F, 2], f32)
            acc = cpool.tile([F, ACC_COLS], f32)
            dummy_d = cpool.tile([F, 1024], f32)  # DVE relu2 main-out sink
            dummy_a = cpool.tile([F, 1024], f32)  # ACT relu2 main-out sink

            # all 8 PSUM banks as one fp32 tensor; 1024 cols = 2 banks
            psum_all = pspool.tile([F, 4096], f32)

            x_all = cpool.tile([F, PER_CORE], bf16)

            # --- input DMAs ------------------------------------------------
            # weights first on the sync ring, first x chunk concurrently on
            # the scalar ring; remaining chunks alternate between rings.
            nc.sync.dma_start(wpk_sb[:], wpk[:])
            nc.sync.dma_start(bpk_sb[:], bpk[:])
            pos = 0
            for ci, w in enumerate(CHUNKS):
                eng = nc.scalar if ci % 2 == 0 else nc.sync
                eng.dma_start(x_all[:, pos : pos + w], xt[:, pos : pos + w])
                pos += w

            w1_sb = wpk_sb[:, 0:F]
            w2_sb = wpk_sb[:, F : 2 * F]
            b1_sb = bpk_sb[:, 0:1]
            b2_sb = bpk_sb[:, 1:2]

            nc.vector.memset(acc[:], 0.0)

            # slot assignment: layer1 -> cols 0..2047, layer2 -> 2048..4095
            def ps1(g, w):
                s = (g % 2) * 1024
                return psum_all[:, s : s + w]

            def ps2(g, w):
                s = 2048 + (g % 2) * 1024
                return psum_all[:, s : s + w]

            h1_tiles = {}
            t_dve = 0.0
            t_act = 0.0

            def relu_pass(ps, bias, outp, accum):
                nonlocal t_dve, t_act
                w = ps.shape[1]
                if t_act + ACT_NS(w) <= t_dve + DVE_NS(w):
                    t_act += ACT_NS(w)
                    nc.scalar.activation(
                        outp,
                        ps,
                        mybir.ActivationFunctionType.Relu,
                        bias=bias,
                        accum_out=accum,
                    )
                    return "act"
                else:
                    t_dve += DVE_NS(w)
                    nc.vector.tensor_scalar(
                        outp,
                        ps,
                        bias,
                        0.0,
                        op0=mybir.AluOpType.add,
                        op1=mybir.AluOpType.max,
                        accum_out=accum,
                    )
                    return "dve"

            def issue_mm1(g):
                w = GROUP_WIDTHS[g]
                s = gstart[g]
                dst = ps1(g, w)
                for j in range(0, w, 512):
                    jw = min(512, w - j)
                    nc.tensor.matmul(
                        dst[:, j : j + jw],
                        w1_sb,
                        x_all[:, s + j : s + j + jw],
                        start=True,
                        stop=True,
                    )

            def issue_relu1(g):
                w = GROUP_WIDTHS[g]
                h1 = h1pool.tile([F, 1024], bf16, tag="h1")
                h1_tiles[g] = h1
                relu_pass(ps1(g, w), b1_sb, h1[:, :w], None)

            def issue_mm2(g):
                w = GROUP_WIDTHS[g]
                dst = ps2(g, w)
                h1 = h1_tiles[g]
                for j in range(0, w, 512):
                    jw = min(512, w - j)
                    nc.tensor.matmul(
                        dst[:, j : j + jw],
                        w2_sb,
                        h1[:, j : j + jw],
                        start=True,
                        stop=True,
                    )

            def issue_relu2(g):
                w = GROUP_WIDTHS[g]
                # main output is a dead sink; accum_out carries the result
                if t_act + ACT_NS(w) <= t_dve + DVE_NS(w):
                    outp = dummy_a[:, :w]
                else:
                    outp = dummy_d[:, :w]
                relu_pass(ps2(g, w), b2_sb, outp, acc[:, g : g + 1])

            # software pipeline over supersteps of 2 groups: issue mm1 for
            # step s together with mm2 for step s-1 so the PE stream only
            # switches stationary weights twice per superstep and never
            # waits on an in-flight relu.
            steps = [
                list(range(i, min(i + 2, N_GROUPS))) for i in range(0, N_GROUPS, 2)
            ]
            for si in range(len(steps) + 1):
                cur = steps[si] if si < len(steps) else []
                prev = steps[si - 1] if si > 0 else []
                for g in cur:
                    issue_mm1(g)
                for g in prev:
                    issue_mm2(g)
                for g in cur:
                    issue_relu1(g)
                for g in prev:
                    issue_relu2(g)

            nc.sync.dma_start(out[:], acc[:])

    nc.compile()
    return nc


def _get_program():
    if "p" not in _COMPILED:
        _COMPILED["p"] = _build_program()
    return _COMPILED["p"]


def _run_on_device(in_maps, **kwargs):
    from concourse.bass_utils import run_bass_kernel_spmd

    nc = _get_program()
    return run_bass_kernel_spmd(nc, in_maps, core_ids=list(range(N_CORES)), **kwargs)


def _make_in_maps(x, W1, b1, W2, b2):
    import ml_dtypes

    dt = np.dtype(ml_dtypes.bfloat16)
    x = np.ascontiguousarray(np.asarray(x, dtype=np.float32)).reshape(N_NODES, F)
    shards = x.reshape(N_CORES, PER_CORE, F)
    wpack = np.concatenate(
        [np.asarray(W1, np.float32).T, np.asarray(W2, np.float32).T], axis=1
    ).astype(dt)
    bpack = np.stack(
        [np.asarray(b1, np.float32), np.asarray(b2, np.float32)], axis=1
    )
    in_maps = []
    for c in range(N_CORES):
        in_maps.append(
            {
                "xt": np.ascontiguousarray(shards[c].T).astype(dt),
                "wpack": wpack,
                "bpack": bpack,
            }
        )
    return in_maps


def _host_head(partials, W3, b3, Wl, bl):
    # partials: [N_CORES, 128, ACC_COLS] fp32 per-group sums of h2.
    g = partials[:, :, :N_GROUPS].astype(np.float64).sum(axis=(0, 2)) / float(N_NODES)
    z = np.asarray(W3, np.float64) @ g + np.asarray(b3, np.float64)
    logits = np.asarray(Wl, np.float64) @ z + np.asarray(bl, np.float64)
    m = logits.max()
    ls = logits - (m + np.log(np.exp(logits - m).sum()))
    return ls[None, :].astype(np.float32)


def kernel(x, edge_index, W1, b1, W2, b2, W3, b3, Wl, bl, **_unused):
    # edge_index is unused by the reference computation (ChebConv K=1).
    in_maps = _make_in_maps(x, W1, b1, W2, b2)
    res = _run_on_device(in_maps)
    partials = np.stack(
        [np.asarray(r["partial"], np.float32) for r in res.results]
    )
    return _host_head(partials, W3, b3, Wl, bl)


# revision 6
# speedup vs baseline: 1.2620x; 1.2620x over previous
"""Trainium2 Bass kernel for nn_GCN_26242250179008.

The reference model is a ChebConv(K=1) stack, which degenerates to plain
dense linear layers (edge_index is never used):

    h = relu(x @ W1.T + b1); h = relu(h @ W2.T + b2); h = h @ W3.T + b3
    g = mean(h, axis=0); out = log_softmax(g @ Wl.T + bl)

Because layer 3 is linear (no relu), mean() commutes with it:
    mean(h3) = mean(h2) @ W3.T + b3
so the device only needs sum_n relu(W2 @ relu(W1 @ x_n + b1) + b2) — a
[128] partial per core.  Layer 3, the classifier head and log_softmax are
O(128^2) and run on host.

Sharding: data-parallel over nodes.  x is split row-wise into 8 shards of
25000 nodes; each shard is transposed on host to [128 features, 25000
nodes] so features sit on SBUF partitions and the matmuls need no
on-device transpose.  The small weights are replicated.  Per-core output
is a [128, 32] fp32 matrix of per-group partial sums; the host reduces
over groups and cores (cheaper than an all-reduce for ~16KB).

Perf notes (HW-measured):
  * Steady state is bound by PSUM evacuation, split across the two
    PSUM-capable engines: ScalarE ACTIVATE(relu) ~(w+181)/1.2 ns without
    accum (+~480ns with accum_out), DVE scalar_tensor_tensor(add,max)
    ~(w+120)/0.96 ns (accum_out free).  So ScalarE takes most relu1
    groups and DVE takes most relu2 groups, with a couple of groups
    swapped to balance total time (~28.5us each).
  * The per-group sums go out as one [128, 32] fp32 DMA (128B per
    partition).  A [128,1] output emits 4-byte descriptors whose HBM
    read-modify-write completion costs ~8us extra at the end.
  * Weights go out first on the sync DGE ring while the first x chunk
    rides the scalar ring, so the first matmul starts ~2.5us after the
    Tile main block opens.
"""

import math
import os

import numpy as np

N_NODES = 200_000
F = 128
N_CORES = 8
PER_CORE = N_NODES // N_CORES  # 25000
GROUP = 1024
MM_N = 512  # one fp32 PSUM bank per matmul output

N_GROUPS = math.ceil(PER_CORE / GROUP)  # 25 (24 full + 424)
ACC_COLS = 32  # padded so the out-DMA moves 128B per partition

# DMA chunks; first ones small so the pipeline fills early, the rest
# ~1MB for full DMA efficiency.
CHUNKS = [1024, 1024, 2048, 4096, 4096, 4096, 4096, 4096, 424]
assert sum(CHUNKS) == PER_CORE

# measured per-pass costs (ns) for [128,w] fp32 PSUM sources
DVE_NS = lambda w: (w + 120) / 0.96  # scalar_tensor_tensor, accum free
ACT_NS = lambda w: (w + 181) / 1.2  # activation, no accum
ACT_ACC_NS = lambda w: (w + 400) / 1.2 + 219  # activation + accum read

_COMPILED = {}


def _build_program():
    from concourse import bacc, mybir, tile

    f32 = mybir.dt.float32
    bf16 = mybir.dt.bfloat16

    nc = bacc.Bacc(None, target_bir_lowering=False, debug=False)

    xt = nc.dram_tensor("xt", [F, PER_CORE], bf16, kind="ExternalInput")
    wpk = nc.dram_tensor("wpack", [F, 2 * F], bf16, kind="ExternalInput")
    bpk = nc.dram_tensor("bpack", [F, 2], f32, kind="ExternalInput")
    out = nc.dram_tensor("partial", [F, ACC_COLS], f32, kind="ExternalOutput")

    # plan the engine split ahead of time with the measured costs:
    # ScalarE prefers relu1 (no accumulator), DVE prefers relu2 (free
    # accumulator); swap a couple of groups to even the finish times.
    widths = [min(GROUP, PER_CORE - g * GROUP) for g in range(N_GROUPS)]

    t_act = sum(ACT_NS(w) for w in widths)  # all relu1 on ACT
    t_dve = sum(DVE_NS(w) for w in widths)  # all relu2 on DVE
    relu2_on_act = set()
    # move relu2 groups (spread through the schedule) from DVE to ACT
    # while it improves the makespan
    candidates = [7, 15, 21, 3, 11, 18]
    for g in candidates:
        w = widths[g]
        if max(t_act + ACT_ACC_NS(w), t_dve - DVE_NS(w)) < max(t_act, t_dve):
            t_act += ACT_ACC_NS(w)
            t_dve -= DVE_NS(w)
            relu2_on_act.add(g)
    relu1_on_dve = set()
    for g in [5, 13, 19, 9, 23]:
        w = widths[g]
        if max(t_dve + DVE_NS(w), t_act - ACT_NS(w)) < max(t_act, t_dve):
            t_dve += DVE_NS(w)
            t_act -= ACT_NS(w)
            relu1_on_dve.add(g)

    with tile.TileContext(nc, pool_alloc_mode="queue") as tc:
        with (
            tc.tile_pool(name="const", bufs=1) as cpool,
            tc.tile_pool(name="h1", bufs=6) as h1pool,
            tc.tile_pool(name="ps1", bufs=2, space="PSUM") as ps1pool,
            tc.tile_pool(name="ps2", bufs=2, space="PSUM") as ps2pool,
        ):
            wpk_sb = cpool.tile([F, 2 * F], bf16)
            bpk_sb = cpool.tile([F, 2], f32)
            acc = cpool.tile([F, ACC_COLS], f32)
            zero_sb = cpool.tile([F, GROUP], f32)
            dummy_d = cpool.tile([F, GROUP], f32)  # DVE relu2 main-out sink
            dummy_a = cpool.tile([F, GROUP], f32)  # ACT relu2 main-out sink
            x_all = cpool.tile([F, PER_CORE], bf16)

            # weights first on the sync ring; first x chunk concurrently on
            # the scalar ring; remaining chunks alternate between rings.
            nc.sync.dma_start(wpk_sb[:], wpk[:])
            nc.sync.dma_start(bpk_sb[:], bpk[:])
            pos = 0
            for ci, w in enumerate(CHUNKS):
                eng = nc.scalar if ci % 2 == 0 else nc.sync
                eng.dma_start(x_all[:, pos : pos + w], xt[:, pos : pos + w])
                pos += w

            w1_sb = wpk_sb[:, 0:F]
            w2_sb = wpk_sb[:, F : 2 * F]
            b1_sb = bpk_sb[:, 0:1]
            b2_sb = bpk_sb[:, 1:2]

            nc.vector.memset(zero_sb[:], 0.0)
            nc.vector.memset(acc[:], 0.0)

            def dve_relu(ps, bias, outp, accum, gw):
                nc.vector.scalar_tensor_tensor(
                    outp,
                    ps[:, :gw],
                    bias,
                    zero_sb[:, :gw],
                    op0=mybir.AluOpType.add,
                    op1=mybir.AluOpType.max,
                    accum_out=accum,
                )

            def act_relu(ps, bias, outp, accum, gw):
                nc.scalar.activation(
                    outp,
                    ps[:, :gw],
                    mybir.ActivationFunctionType.Relu,
                    bias=bias,
                    accum_out=accum,
                )

            for g in range(N_GROUPS):
                gw = widths[g]
                start = g * GROUP
                ps1 = ps1pool.tile([F, GROUP], f32, tag="ps1")
                for j in range(0, gw, MM_N):
                    jw = min(MM_N, gw - j)
                    nc.tensor.matmul(
                        ps1[:, j : j + jw],
                        w1_sb,
                        x_all[:, start + j : start + j + jw],
                        start=True,
                        stop=True,
                    )
                h1 = h1pool.tile([F, GROUP], bf16, tag="h1")
                if g in relu1_on_dve:
                    dve_relu(ps1, b1_sb, h1[:, :gw], None, gw)
                else:
                    act_relu(ps1, b1_sb, h1[:, :gw], None, gw)

                ps2 = ps2pool.tile([F, GROUP], f32, tag="ps2")
                for j in range(0, gw, MM_N):
                    jw = min(MM_N, gw - j)
                    nc.tensor.matmul(
                        ps2[:, j : j + jw],
                        w2_sb,
                        h1[:, j : j + jw],
                        start=True,
                        stop=True,
                    )
                accum = acc[:, g : g + 1]
                if g in relu2_on_act:
                    act_relu(ps2, b2_sb, dummy_a[:, :gw], accum, gw)
                else:
                    dve_relu(ps2, b2_sb, dummy_d[:, :gw], accum, gw)

            nc.sync.dma_start(out[:], acc[:])

    nc.compile()
    return nc


def _get_program():
    if "p" not in _COMPILED:
        _COMPILED["p"] = _build_program()
    return _COMPILED["p"]


def _run_on_device(in_maps, **kwargs):
    from concourse.bass_utils import run_bass_kernel_spmd

    nc = _get_program()
    return run_bass_kernel_spmd(nc, in_maps, core_ids=list(range(N_CORES)), **kwargs)


def _make_in_maps(x, W1, b1, W2, b2):
    import ml_dtypes

    dt = np.dtype(ml_dtypes.bfloat16)
    x = np.ascontiguousarray(np.asarray(x, dtype=np.float32)).reshape(N_NODES, F)
    shards = x.reshape(N_CORES, PER_CORE, F)
    wpack = np.concatenate(
        [np.asarray(W1, np.float32).T, np.asarray(W2, np.float32).T], axis=1
    ).astype(dt)
    bpack = np.stack(
        [np.asarray(b1, np.float32), np.asarray(b2, np.float32)], axis=1
    )
    in_maps = []
    for c in range(N_CORES):
        in_maps.append(
            {
                "xt": np.ascontiguousarray(shards[c].T).astype(dt),
                "wpack": wpack,
                "bpack": bpack,
            }
        )
    return in_maps


def _host_head(partials, W3, b3, Wl, bl):
    # partials: [N_CORES, 128, ACC_COLS] fp32 per-group sums of h2.
    g = partials[:, :, :N_GROUPS].astype(np.float64).sum(axis=(0, 2)) / float(N_NODES)
    z = np.asarray(W3, np.float64) @ g + np.asarray(b3, np.float64)
    logits = np.asarray(Wl, np.float64) @ z + np.asarray(bl, np.float64)
    m = logits.max()
    ls = logits - (m + np.log(np.exp(logits - m).sum()))
    return ls[None, :].astype(np.float32)


def kernel(x, edge_index, W1, b1, W2, b2, W3, b3, Wl, bl, **_unused):
    # edge_index is unused by the reference computation (ChebConv K=1).
    in_maps = _make_in_maps(x, W1, b1, W2, b2)
    res = _run_on_device(in_maps)
    partials = np.stack(
        [np.asarray(r["partial"], np.float32) for r in res.results]
    )
    return _host_head(partials, W3, b3, Wl, bl)


# revision 13
# speedup vs baseline: 1.4186x; 1.1241x over previous
"""Trainium2 Bass kernel for nn_GCN_26242250179008.

The reference model is a ChebConv(K=1) stack, which degenerates to plain
dense linear layers (edge_index is never used):

    h = relu(x @ W1.T + b1); h = relu(h @ W2.T + b2); h = h @ W3.T + b3
    g = mean(h, axis=0); out = log_softmax(g @ Wl.T + bl)

Because layer 3 is linear (no relu), mean() commutes with it:
    mean(h3) = mean(h2) @ W3.T + b3
so the device only needs sum_n relu(W2 @ relu(W1 @ x_n + b1) + b2) — a
[128] partial per core.  Layer 3, the classifier head and log_softmax are
O(128^2) and run on host.

Sharding: data-parallel over nodes.  x is split row-wise into 8 shards of
25000 nodes; each shard is transposed on host to [128 features, 25000
nodes] so features sit on SBUF partitions and the matmuls need no
on-device transpose.  The small weights are replicated.  Per-core output
is a [128, 32] fp32 matrix of per-group partial sums; the host reduces
over groups and cores (cheaper than an all-reduce for ~16KB).

Perf notes (HW-measured):
  * Steady state is bound by PSUM evacuation, split across the two
    PSUM-capable engines: ScalarE ACTIVATE(relu) ~(w+181)/1.2 ns without
    accum (+~480ns with accum_out), DVE scalar_tensor_tensor(add,max)
    ~(w+120)/0.96 ns (accum_out free).  So ScalarE takes most relu1
    groups and DVE takes most relu2 groups, with a couple of groups
    swapped to balance total time (~28.5us each).
  * The per-group sums go out as one [128, 32] fp32 DMA (128B per
    partition).  A [128,1] output emits 4-byte descriptors whose HBM
    read-modify-write completion costs ~8us extra at the end.
  * Weights go out first on the sync DGE ring while the first x chunk
    rides the scalar ring, so the first matmul starts ~2.5us after the
    Tile main block opens.
"""

import math
import os

import numpy as np

N_NODES = 200_000
F = 128
N_CORES = 8
PER_CORE = N_NODES // N_CORES  # 25000
GROUP = 1024
MM_N = 512  # one fp32 PSUM bank per matmul output

N_GROUPS = math.ceil(PER_CORE / GROUP)  # 25 (24 full + 424)
ACC_COLS = 32  # padded so the out-DMA moves 128B per partition

# DMA chunks; first ones small so the pipeline fills early, the rest
# ~1MB for full DMA efficiency.
CHUNKS = [1024, 1024, 1024, 1024, 2048, 2048, 4096, 4096, 4096, 4096, 424]
assert sum(CHUNKS) == PER_CORE

# measured per-pass costs (ns) for [128,w] fp32 PSUM sources
DVE_NS = lambda w: (w + 140) / 0.96 + 9  # scalar_tensor_tensor, accum free
ACT_NS = lambda w: (w + 200) / 1.2  # activation, no accum
ACT_ACC_NS = lambda w: (w + 200) / 1.2 + 470  # activation + accum read

_COMPILED = {}


def _build_program():
    from concourse import bacc, mybir, tile

    f32 = mybir.dt.float32
    bf16 = mybir.dt.bfloat16

    nc = bacc.Bacc(None, target_bir_lowering=False, debug=False)

    xt = nc.dram_tensor("xt", [F, PER_CORE], bf16, kind="ExternalInput")
    wpk = nc.dram_tensor("wpack", [F, 2 * F], bf16, kind="ExternalInput")
    bpk = nc.dram_tensor("bpack", [F, 2], f32, kind="ExternalInput")
    out = nc.dram_tensor("partial", [F, ACC_COLS], f32, kind="ExternalOutput")

    # plan the engine split ahead of time with the measured costs:
    # ScalarE prefers relu1 (no accumulator), DVE prefers relu2 (free
    # accumulator); swap a couple of groups to even the finish times.
    widths = [min(GROUP, PER_CORE - g * GROUP) for g in range(N_GROUPS)]

    t_act = sum(ACT_NS(w) for w in widths)  # all relu1 on ACT
    t_dve = sum(DVE_NS(w) for w in widths)  # all relu2 on DVE
    relu2_on_act = set()
    # move relu2 groups (spread through the schedule) from DVE to ACT
    # while it improves the makespan
    candidates = [8, 16, 20, 4, 12, 22]
    for g in candidates:
        w = widths[g]
        if max(t_act + ACT_ACC_NS(w), t_dve - DVE_NS(w)) < max(t_act, t_dve):
            t_act += ACT_ACC_NS(w)
            t_dve -= DVE_NS(w)
            relu2_on_act.add(g)
    relu1_on_dve = set()
    for g in [5, 13, 19, 9, 23]:
        w = widths[g]
        if max(t_dve + DVE_NS(w), t_act - ACT_NS(w)) < max(t_act, t_dve):
            t_dve += DVE_NS(w)
            t_act -= ACT_NS(w)
            relu1_on_dve.add(g)

    with tile.TileContext(nc, pool_alloc_mode="queue") as tc:
        with (
            tc.tile_pool(name="const", bufs=1) as cpool,
            tc.tile_pool(name="h1", bufs=6) as h1pool,
            tc.tile_pool(name="h2", bufs=4) as h2pool,
            tc.tile_pool(name="ps1", bufs=2, space="PSUM") as ps1pool,
            tc.tile_pool(name="ps2", bufs=2, space="PSUM") as ps2pool,
        ):
            wpk_sb = cpool.tile([F, 2 * F], bf16)
            bpk_sb = cpool.tile([F, 2], f32)
            acc = cpool.tile([F, ACC_COLS], f32)
            zero_sb = cpool.tile([F, GROUP], f32)
            x_all = cpool.tile([F, PER_CORE], bf16)

            # first x chunk on the scalar ring so it transfers in parallel
            # with the sync ring's weights + later chunks (mirrors the DGE
            # ring split; keeping bulk chunks off the scalar ring keeps its
            # descriptor generation away from the ScalarE relu pipeline).
            nc.sync.dma_start(wpk_sb[:], wpk[:])
            nc.sync.dma_start(bpk_sb[:], bpk[:])
            pos = 0
            for ci, w in enumerate(CHUNKS):
                eng = nc.scalar if ci == 0 else nc.sync
                eng.dma_start(x_all[:, pos : pos + w], xt[:, pos : pos + w])
                pos += w

            w1_sb = wpk_sb[:, 0:F]
            w2_sb = wpk_sb[:, F : 2 * F]
            b1_sb = bpk_sb[:, 0:1]
            b2_sb = bpk_sb[:, 1:2]

            nc.vector.memset(zero_sb[:], 0.0)
            nc.vector.memset(acc[:], 0.0)

            def dve_relu(ps, bias, outp, accum, gw):
                nc.vector.scalar_tensor_tensor(
                    outp,
                    ps[:, :gw],
                    bias,
                    zero_sb[:, :gw],
                    op0=mybir.AluOpType.add,
                    op1=mybir.AluOpType.max,
                    accum_out=accum,
                )

            def act_relu(ps, bias, outp, accum, gw):
                nc.scalar.activation(
                    outp,
                    ps[:, :gw],
                    mybir.ActivationFunctionType.Relu,
                    bias=bias,
                    accum_out=accum,
                )

            # issue per pair of groups so the PE stream batches stationary
            # weights: mm1(g) mm1(g+1) [W1 once], relu1s, mm2(g) mm2(g+1)
            # [W2 once], relu2s — halves the LDWEIGHTS count.
            def issue_mm(dst, wsb, src, gw):
                for j in range(0, gw, MM_N):
                    jw = min(MM_N, gw - j)
                    nc.tensor.matmul(
                        dst[:, j : j + jw],
                        wsb,
                        src[:, j : j + jw],
                        start=True,
                        stop=True,
                    )

            for g0 in range(0, N_GROUPS, 2):
                pair = [g for g in (g0, g0 + 1) if g < N_GROUPS]
                ps1s, h1s, ps2s = {}, {}, {}
                for g in pair:
                    gw = widths[g]
                    ps1s[g] = ps1pool.tile([F, GROUP], f32, tag="ps1", name=f"ps1_{g}")
                    issue_mm(ps1s[g], w1_sb, x_all[:, g * GROUP : g * GROUP + gw], gw)
                for g in pair:
                    gw = widths[g]
                    h1s[g] = h1pool.tile([F, GROUP], bf16, tag="h1", name=f"h1_{g}")
                    if g in relu1_on_dve:
                        dve_relu(ps1s[g], b1_sb, h1s[g][:, :gw], None, gw)
                    else:
                        act_relu(ps1s[g], b1_sb, h1s[g][:, :gw], None, gw)
                for g in pair:
                    gw = widths[g]
                    ps2s[g] = ps2pool.tile([F, GROUP], f32, tag="ps2", name=f"ps2_{g}")
                    issue_mm(ps2s[g], w2_sb, h1s[g][:, :gw], gw)
                for g in pair:
                    gw = widths[g]
                    accum = acc[:, g : g + 1]
                    h2 = h2pool.tile([F, GROUP], f32, tag="h2")
                    if g in relu2_on_act:
                        act_relu(ps2s[g], b2_sb, h2[:, :gw], accum, gw)
                    else:
                        dve_relu(ps2s[g], b2_sb, h2[:, :gw], accum, gw)

            nc.sync.dma_start(out[:], acc[:])

    nc.compile()
    return nc


def _get_program():
    if "p" not in _COMPILED:
        _COMPILED["p"] = _build_program()
    return _COMPILED["p"]


def _run_on_device(in_maps, **kwargs):
    from concourse.bass_utils import run_bass_kernel_spmd

    nc = _get_program()
    return run_bass_kernel_spmd(nc, in_maps, core_ids=list(range(N_CORES)), **kwargs)


def _make_in_maps(x, W1, b1, W2, b2):
    import ml_dtypes

    dt = np.dtype(ml_dtypes.bfloat16)
    x = np.ascontiguousarray(np.asarray(x, dtype=np.float32)).reshape(N_NODES, F)
    shards = x.reshape(N_CORES, PER_CORE, F)
    wpack = np.concatenate(
        [np.asarray(W1, np.float32).T, np.asarray(W2, np.float32).T], axis=1
    ).astype(dt)
    bpack = np.stack(
        [np.asarray(b1, np.float32), np.asarray(b2, np.float32)], axis=1
    )
    in_maps = []
    for c in range(N_CORES):
        in_maps.append(
            {
                "xt": np.ascontiguousarray(shards[c].T).astype(dt),
                "wpack": wpack,
                "bpack": bpack,
            }
        )
    return in_maps


def _host_head(partials, W3, b3, Wl, bl):
    # partials: [N_CORES, 128, ACC_COLS] fp32 per-group sums of h2.
    g = partials[:, :, :N_GROUPS].astype(np.float64).sum(axis=(0, 2)) / float(N_NODES)
    z = np.asarray(W3, np.float64) @ g + np.asarray(b3, np.float64)
    logits = np.asarray(Wl, np.float64) @ z + np.asarray(bl, np.float64)
    m = logits.max()
    ls = logits - (m + np.log(np.exp(logits - m).sum()))
    return ls[None, :].astype(np.float32)


def kernel(x, edge_index, W1, b1, W2, b2, W3, b3, Wl, bl, **_unused):
    # edge_index is unused by the reference computation (ChebConv K=1).
    in_maps = _make_in_maps(x, W1, b1, W2, b2)
    res = _run_on_device(in_maps)
    partials = np.stack(
        [np.asarray(r["partial"], np.float32) for r in res.results]
    )
    return _host_head(partials, W3, b3, Wl, bl)


# revision 14
# speedup vs baseline: 1.4566x; 1.0268x over previous
"""Trainium2 Bass kernel for nn_GCN_26242250179008.

The reference model is a ChebConv(K=1) stack, which degenerates to plain
dense linear layers (edge_index is never used):

    h = relu(x @ W1.T + b1); h = relu(h @ W2.T + b2); h = h @ W3.T + b3
    g = mean(h, axis=0); out = log_softmax(g @ Wl.T + bl)

Because layer 3 is linear (no relu), mean() commutes with it:
    mean(h3) = mean(h2) @ W3.T + b3
so the device only needs sum_n relu(W2 @ relu(W1 @ x_n + b1) + b2) — a
[128] partial per core.  Layer 3, the classifier head and log_softmax are
O(128^2) and run on host.

Sharding: data-parallel over nodes.  x is split row-wise into 8 shards of
25000 nodes; each shard is transposed on host to [128 features, 25000
nodes] so features sit on SBUF partitions and the matmuls need no
on-device transpose.  The small weights are replicated.  Per-core output
is a [128, 32] fp32 matrix of per-group partial sums; the host reduces
over groups and cores (cheaper than an all-reduce for ~16KB).

Perf notes (HW-measured):
  * Steady state is bound by PSUM evacuation, split across the two
    PSUM-capable engines: ScalarE ACTIVATE(relu) ~(w+181)/1.2 ns without
    accum (+~480ns with accum_out), DVE scalar_tensor_tensor(add,max)
    ~(w+120)/0.96 ns (accum_out free).  So ScalarE takes most relu1
    groups and DVE takes most relu2 groups, with a couple of groups
    swapped to balance total time (~28.5us each).
  * The per-group sums go out as one [128, 32] fp32 DMA (128B per
    partition).  A [128,1] output emits 4-byte descriptors whose HBM
    read-modify-write completion costs ~8us extra at the end.
  * Weights go out first on the sync DGE ring while the first x chunk
    rides the scalar ring, so the first matmul starts ~2.5us after the
    Tile main block opens.
"""

import math
import os

import numpy as np

N_NODES = 200_000
F = 128
N_CORES = 8
PER_CORE = N_NODES // N_CORES  # 25000
GROUP = 1024
MM_N = 512  # one fp32 PSUM bank per matmul output

# two 512 warm-up groups so compute starts on the first small DMA chunk
GROUP_WIDTHS = [512, 512] + [1024] * 23 + [424]
assert sum(GROUP_WIDTHS) == PER_CORE
N_GROUPS = len(GROUP_WIDTHS)  # 26
ACC_COLS = 32  # padded so the out-DMA moves 128B per partition

# DMA chunks; first ones small so the pipeline fills early, the rest
# ~1MB for full DMA efficiency.
CHUNKS = [512, 512, 1024, 1024, 1024, 2048, 2048, 4096, 4096, 4096, 4096, 424]
assert sum(CHUNKS) == PER_CORE

# measured per-pass costs (ns) for [128,w] fp32 PSUM sources
DVE_NS = lambda w: (w + 125) / 0.96 + 9  # scalar_tensor_tensor, accum free
ACT_NS = lambda w: (w + 230) / 1.2  # activation, no accum
ACT_ACC_NS = lambda w: (w + 230) / 1.2 + 500  # activation + accum read

_COMPILED = {}


def _build_program():
    from concourse import bacc, mybir, tile

    f32 = mybir.dt.float32
    bf16 = mybir.dt.bfloat16

    nc = bacc.Bacc(None, target_bir_lowering=False, debug=False)

    xt = nc.dram_tensor("xt", [F, PER_CORE], bf16, kind="ExternalInput")
    wpk = nc.dram_tensor("wpack", [F, 2 * F], bf16, kind="ExternalInput")
    bpk = nc.dram_tensor("bpack", [F, 2], f32, kind="ExternalInput")
    out = nc.dram_tensor("partial", [F, ACC_COLS], f32, kind="ExternalOutput")

    # plan the engine split ahead of time with the measured costs:
    # ScalarE prefers relu1 (no accumulator), DVE prefers relu2 (free
    # accumulator); swap a couple of groups to even the finish times.
    widths = GROUP_WIDTHS
    gstart = [sum(widths[:g]) for g in range(N_GROUPS)]

    t_act = sum(ACT_NS(w) for w in widths)  # all relu1 on ACT
    t_dve = sum(DVE_NS(w) for w in widths)  # all relu2 on DVE
    relu2_on_act = set()
    # move relu2 groups (spread through the schedule) from DVE to ACT
    # while it improves the makespan
    candidates = [13, 19, 7, 23, 10, 16]
    for g in candidates:
        w = widths[g]
        if max(t_act + ACT_ACC_NS(w), t_dve - DVE_NS(w)) < max(t_act, t_dve):
            t_act += ACT_ACC_NS(w)
            t_dve -= DVE_NS(w)
            relu2_on_act.add(g)
    relu1_on_dve = set()
    for g in [6, 14, 20, 10, 24]:
        w = widths[g]
        if max(t_dve + DVE_NS(w), t_act - ACT_NS(w)) < max(t_act, t_dve):
            t_dve += DVE_NS(w)
            t_act -= ACT_NS(w)
            relu1_on_dve.add(g)

    with tile.TileContext(nc, pool_alloc_mode="queue") as tc:
        with (
            tc.tile_pool(name="const", bufs=1) as cpool,
            tc.tile_pool(name="h1", bufs=6) as h1pool,
            tc.tile_pool(name="h2", bufs=4) as h2pool,
            tc.tile_pool(name="ps1", bufs=2, space="PSUM") as ps1pool,
            tc.tile_pool(name="ps2", bufs=2, space="PSUM") as ps2pool,
        ):
            wpk_sb = cpool.tile([F, 2 * F], bf16)
            bpk_sb = cpool.tile([F, 2], f32)
            acc = cpool.tile([F, ACC_COLS], f32)
            zero_sb = cpool.tile([F, GROUP], f32)
            x_all = cpool.tile([F, PER_CORE], bf16)

            # first x chunk on the scalar ring so it transfers in parallel
            # with the sync ring's weights + later chunks (mirrors the DGE
            # ring split; keeping bulk chunks off the scalar ring keeps its
            # descriptor generation away from the ScalarE relu pipeline).
            nc.sync.dma_start(wpk_sb[:], wpk[:])
            nc.sync.dma_start(bpk_sb[:], bpk[:])
            pos = 0
            for ci, w in enumerate(CHUNKS):
                eng = nc.scalar if ci == 0 else nc.sync
                eng.dma_start(x_all[:, pos : pos + w], xt[:, pos : pos + w])
                pos += w

            w1_sb = wpk_sb[:, 0:F]
            w2_sb = wpk_sb[:, F : 2 * F]
            b1_sb = bpk_sb[:, 0:1]
            b2_sb = bpk_sb[:, 1:2]

            nc.vector.memset(zero_sb[:], 0.0)
            nc.vector.memset(acc[:], 0.0)

            def dve_relu(ps, bias, outp, accum, gw):
                nc.vector.scalar_tensor_tensor(
                    outp,
                    ps[:, :gw],
                    bias,
                    zero_sb[:, :gw],
                    op0=mybir.AluOpType.add,
                    op1=mybir.AluOpType.max,
                    accum_out=accum,
                )

            def act_relu(ps, bias, outp, accum, gw):
                nc.scalar.activation(
                    outp,
                    ps[:, :gw],
                    mybir.ActivationFunctionType.Relu,
                    bias=bias,
                    accum_out=accum,
                )

            # issue per pair of groups so the PE stream batches stationary
            # weights: mm1(g) mm1(g+1) [W1 once], relu1s, mm2(g) mm2(g+1)
            # [W2 once], relu2s — halves the LDWEIGHTS count.
            def issue_mm(dst, wsb, src, gw):
                for j in range(0, gw, MM_N):
                    jw = min(MM_N, gw - j)
                    nc.tensor.matmul(
                        dst[:, j : j + jw],
                        wsb,
                        src[:, j : j + jw],
                        start=True,
                        stop=True,
                    )

            for g0 in range(0, N_GROUPS, 2):
                pair = [g for g in (g0, g0 + 1) if g < N_GROUPS]
                ps1s, h1s, ps2s = {}, {}, {}
                for g in pair:
                    gw = widths[g]
                    ps1s[g] = ps1pool.tile([F, GROUP], f32, tag="ps1", name=f"ps1_{g}")
                    issue_mm(ps1s[g], w1_sb, x_all[:, gstart[g] : gstart[g] + gw], gw)
                for g in pair:
                    gw = widths[g]
                    h1s[g] = h1pool.tile([F, GROUP], bf16, tag="h1", name=f"h1_{g}")
                    if g in relu1_on_dve:
                        dve_relu(ps1s[g], b1_sb, h1s[g][:, :gw], None, gw)
                    else:
                        act_relu(ps1s[g], b1_sb, h1s[g][:, :gw], None, gw)
                for g in pair:
                    gw = widths[g]
                    ps2s[g] = ps2pool.tile([F, GROUP], f32, tag="ps2", name=f"ps2_{g}")
                    issue_mm(ps2s[g], w2_sb, h1s[g][:, :gw], gw)
                for g in pair:
                    gw = widths[g]
                    accum = acc[:, g : g + 1]
                    h2 = h2pool.tile([F, GROUP], f32, tag="h2")
                    if g in relu2_on_act:
                        act_relu(ps2s[g], b2_sb, h2[:, :gw], accum, gw)
                    else:
                        dve_relu(ps2s[g], b2_sb, h2[:, :gw], accum, gw)

            nc.sync.dma_start(out[:], acc[:])

    nc.compile()
    return nc


def _get_program():
    if "p" not in _COMPILED:
        _COMPILED["p"] = _build_program()
    return _COMPILED["p"]


def _run_on_device(in_maps, **kwargs):
    from concourse.bass_utils import run_bass_kernel_spmd

    nc = _get_program()
    return run_bass_kernel_spmd(nc, in_maps, core_ids=list(range(N_CORES)), **kwargs)


def _make_in_maps(x, W1, b1, W2, b2):
    import ml_dtypes

    dt = np.dtype(ml_dtypes.bfloat16)
    x = np.ascontiguousarray(np.asarray(x, dtype=np.float32)).reshape(N_NODES, F)
    shards = x.reshape(N_CORES, PER_CORE, F)
    wpack = np.concatenate(
        [np.asarray(W1, np.float32).T, np.asarray(W2, np.float32).T], axis=1
    ).astype(dt)
    bpack = np.stack(
        [np.asarray(b1, np.float32), np.asarray(b2, np.float32)], axis=1
    )
    in_maps = []
    for c in range(N_CORES):
        in_maps.append(
            {
                "xt": np.ascontiguousarray(shards[c].T).astype(dt),
                "wpack": wpack,
                "bpack": bpack,
            }
        )
    return in_maps


def _host_head(partials, W3, b3, Wl, bl):
    # partials: [N_CORES, 128, ACC_COLS] fp32 per-group sums of h2.
    g = partials[:, :, :N_GROUPS].astype(np.float64).sum(axis=(0, 2)) / float(N_NODES)
    z = np.asarray(W3, np.float64) @ g + np.asarray(b3, np.float64)
    logits = np.asarray(Wl, np.float64) @ z + np.asarray(bl, np.float64)
    m = logits.max()
    ls = logits - (m + np.log(np.exp(logits - m).sum()))
    return ls[None, :].astype(np.float32)


def kernel(x, edge_index, W1, b1, W2, b2, W3, b3, Wl, bl, **_unused):
    # edge_index is unused by the reference computation (ChebConv K=1).
    in_maps = _make_in_maps(x, W1, b1, W2, b2)
    res = _run_on_device(in_maps)
    partials = np.stack(
        [np.asarray(r["partial"], np.float32) for r in res.results]
    )
    return _host_head(partials, W3, b3, Wl, bl)
